# revision 6
# baseline (speedup 1.0000x reference)
"""Sliding-window GQA causal self-attention on 8 TRN2 NeuronCores.

Sharding: sequence-parallel. B=2 batches x 4 chunks of 512 tokens = 8 shards.
Sliding window (512) makes attention local: each chunk only needs the 512
preceding tokens (halo) for K/V, so there are NO collectives. Each core:
  qkv proj (bf16 matmul) -> rotate-half RoPE + RMS-norm -> banded attention
  (scores computed transposed so exp'd probs feed attn@V directly as lhsT,
   softmax denominators fused via a ones-column in V) -> output proj.

v2 layout/schedule:
  - all transposes go through the XBAR DMA-transpose (PE transposes removed)
  - weight/x loads are 512KB-chunk DMAs split across both HWDGE queues
    (sync + scalar); matmuls are k-chunk-major so the PE chases arrivals
  - column order Q0, K, V, Q1..Q3 with rope + attention interleaved per
    4-head group; out proj last with wp prefetched via gpsimd during attn
  - RoPE tables resident in SBUF; Q rows reuse K table rows 4..8
"""

import sys

sys.path.insert(0, "/opt/trn_rl_repo")

import numpy as np
import ml_dtypes

import concourse.bass as bass
import concourse.mybir as mybir
import concourse.tile as tile
from concourse import bacc
from concourse.bass_utils import run_bass_kernel_spmd


def _install_ntff_hook():
    """antenv.axon_hooks is absent on this image; bridge the ctypes NTFF
    profiling hook from trn_agent_boot so trace=True works."""
    import types
    try:
        import antenv.axon_hooks  # noqa: F401
        return
    except ImportError:
        pass
    try:
        from trn_agent_boot.trn_boot import _ntff_profile_via_ctypes
        hook = _ntff_profile_via_ctypes("/opt/axon/libaxon_pjrt.so")
    except Exception:
        hook = None
    mod = types.ModuleType("antenv.axon_hooks")
    mod.get_axon_ntff_profile_hook = lambda: hook
    mod.set_axon_ntff_profile_hook = lambda h: None
    sys.modules["antenv.axon_hooks"] = mod


_install_ntff_hook()

BF16 = mybir.dt.bfloat16
F32 = mybir.dt.float32

B, T, C = 2, 2048, 2048
HQ, HKV, DH, WIN = 16, 4, 128, 512
RMS_EPS = 1.1920929e-07
CHUNK = 512          # own tokens per core
WTOK = 1024          # window tokens per core (halo 512 + own 512)
NKT = WTOK // 128    # 8 k-tiles
NQT = CHUNK // 128   # 4 q-tiles
SCALE = 1.0 / np.sqrt(DH)

_NC_CACHE = [None]


def _build_nc():
    nc = bacc.Bacc("TRN2", target_bir_lowering=False, debug=False, num_devices=8)

    xt = nc.declare_dram_parameter("xt", [C, WTOK], BF16, False)        # x^T window
    wq = nc.declare_dram_parameter("wq", [C, (HQ + 2 * HKV) * DH], BF16, False)
    wp = nc.declare_dram_parameter("wp", [C, C], BF16, False)
    csk = nc.declare_dram_parameter("csk", [WTOK, HKV * DH], BF16, False)
    ssk = nc.declare_dram_parameter("ssk", [WTOK, HKV * DH], BF16, False)
    mask = nc.declare_dram_parameter("mask", [NQT, 128, 256], BF16, False)
    vld = nc.declare_dram_parameter("vld", [128, NKT], BF16, False)
    out = nc.declare_dram_parameter("out", [CHUNK, C], F32, True)

    xt_r = xt.ap().rearrange("(k p) t -> p k t", p=128)       # [128, 16, 1024]
    wq_r = wq.ap().rearrange("(k p) d -> p k d", p=128)       # [128, 16, 3072]
    wp_r = wp.ap().rearrange("(k p) d -> p k d", p=128)       # [128, 16, 2048]
    csk_r = csk.ap().rearrange("(n p) d -> n p d", p=128)     # 8 x [128, 512]
    ssk_r = ssk.ap().rearrange("(n p) d -> n p d", p=128)
    mask_r = mask.ap()

    with tile.TileContext(nc) as tc:
        with (
            tc.tile_pool(name="singles", bufs=1) as singles,
            tc.tile_pool(name="wtiles", bufs=3) as wtiles,      # streamed weight cols
            tc.tile_pool(name="work", bufs=2) as work,          # small temps
            tc.tile_pool(name="ptile", bufs=4) as ptile,        # exp'd probs
            tc.tile_pool(name="psS", bufs=2, space="PSUM") as psS,    # scores (2x2 banks)
            tc.tile_pool(name="psP", bufs=2, space="PSUM") as psP,    # proj (2x1 bank)
            tc.tile_pool(name="psV", bufs=2, space="PSUM") as psV,    # AV (2x1 bank)
        ):
            eps_t = singles.tile([128, 1], F32)
            nc.vector.memset(eps_t, RMS_EPS)

            # resident tensors
            xts = singles.tile([128, 16, WTOK], BF16)           # x^T (C-tile, tok)
            csk_sb = singles.tile([128, NKT, HKV, DH], BF16)    # cos (rows = window)
            ssk_sb = singles.tile([128, NKT, HKV, DH], BF16)
            msk = singles.tile([128, NQT, 2, 128], BF16)
            vldsb = singles.tile([128, NKT], BF16)
            qt = singles.tile([128, HQ, NQT, 128], BF16)        # Q^T [dh, (h, qi, tok)]
            kt = singles.tile([128, HKV, NKT, 128], BF16)
            yt = singles.tile([128, HQ, NQT, 128], BF16)
            vext = [singles.tile([128, HKV, 132], BF16, name=f"vext{j}")
                    for j in range(NKT)]

            # small startup loads on gpsimd (SWDGE queue, off the HWDGE rings)
            for n in range(NKT):
                nc.gpsimd.dma_start(out=csk_sb[:, n, :, :].rearrange("p a b -> p (a b)"),
                                    in_=csk_r[n])
                nc.gpsimd.dma_start(out=ssk_sb[:, n, :, :].rearrange("p a b -> p (a b)"),
                                    in_=ssk_r[n])
            for i in range(NQT):
                nc.gpsimd.dma_start(out=msk[:, i, :, :].rearrange("p a b -> p (a b)"),
                                    in_=mask_r[i])
            nc.gpsimd.dma_start(out=vldsb, in_=vld.ap())
            for j in range(NKT):
                for g in range(HKV):
                    nc.gpsimd.tensor_copy(vext[j][:, g, 128:129], vldsb[:, j:j + 1])

            HW = [nc.sync, nc.scalar]   # the two HWDGE queues

            def load_col(src_r, n, engines, wname):
                wcol = wtiles.tile([128, 16, 512], BF16, tag="w", name=wname)
                for c in range(4):
                    engines[c].dma_start(
                        out=wcol[:, 4 * c:4 * c + 4, :],
                        in_=src_r[:, 4 * c:4 * c + 4, n * 512:(n + 1) * 512])
                return wcol

            def proj_col(wcol, itiles, evac):
                # groups of 2 itiles, k-chunk-major so MMs chase DMA chunks
                for g0 in range(0, len(itiles), 2):
                    grp = itiles[g0:g0 + 2]
                    pps = [psP.tile([128, 512], F32, tag="pp", name="pp") for _ in grp]
                    for c in range(4):
                        for k in range(4 * c, 4 * c + 4):
                            for pi, i in enumerate(grp):
                                nc.tensor.matmul(
                                    pps[pi], xts[:, k, i * 128:(i + 1) * 128],
                                    wcol[:, k, :], start=(k == 0), stop=(k == 15))
                    for pi, i in enumerate(grp):
                        evac(i, pps[pi])

            def rope4(src4, row, dst):
                # src4: [128, 4, DH] raw (pre-rope); dst: [128, 4, 128] slice of
                # qt/kt getting the roped+normalized TRANSPOSED result.
                sq = work.tile([128, 4, DH], BF16, tag="sq")
                nc.vector.tensor_mul(sq, src4, src4)
                ssum = work.tile([128, 4], F32, tag="ssum")
                nc.vector.reduce_sum(out=ssum, in_=sq, axis=mybir.AxisListType.X)
                rstd = work.tile([128, 4], F32, tag="rstd")
                nc.scalar.activation(rstd, ssum, mybir.ActivationFunctionType.Sqrt,
                                     bias=eps_t[:], scale=1.0 / DH)
                rinv = work.tile([128, 4], F32, tag="rinv")
                nc.vector.reciprocal(rinv, rstd)

                cs = csk_sb[:, row, :, :]
                ss = ssk_sb[:, row, :, :]
                tmp = work.tile([128, 4, DH], BF16, tag="tmp")
                ro = work.tile([128, 4, DH], BF16, tag="ro")
                nc.vector.tensor_mul(tmp[:, :, 0:64], src4[:, :, 64:128], ss[:, :, 0:64])
                nc.vector.tensor_mul(tmp[:, :, 64:128], src4[:, :, 0:64], ss[:, :, 64:128])
                nc.vector.tensor_mul(ro, src4, cs)
                nc.vector.tensor_add(ro, ro, tmp)
                for hh in range(4):
                    nc.vector.tensor_scalar_mul(ro[:, hh, :], ro[:, hh, :],
                                                rinv[:, hh:hh + 1])
                nc.scalar.dma_start_transpose(out=dst, in_=ro)

            # evacuation hooks -----------------------------------------------
            def evac_q(n):
                def f(i, pp):
                    qi = i - 4
                    src4 = work.tile([128, 4, DH], BF16, tag="qraw", name="qraw")
                    nc.scalar.copy(out=src4.rearrange("p a b -> p (a b)"), in_=pp[:])
                    rope4(src4, 4 + qi, qt[:, 4 * n:4 * n + 4, qi, :])
                return f

            def evac_k(i, pp):
                src4 = work.tile([128, 4, DH], BF16, tag="kraw", name="kraw")
                nc.scalar.copy(out=src4.rearrange("p a b -> p (a b)"), in_=pp[:])
                rope4(src4, i, kt[:, :, i, :])

            def evac_v(i, pp):
                for g in range(HKV):
                    nc.scalar.copy(out=vext[i][:, g, 0:128],
                                   in_=pp[:, g * 128:(g + 1) * 128])

            # ---------------- attention (one 4-head group) ----------------
            def attention(n):
                ysb4s = [work.tile([128, 4, 128], BF16, tag=f"ysb{qi}",
                                   name=f"ysb{qi}") for qi in range(NQT)]
                g = n  # kv group for heads 4n..4n+3
                for hh in range(4):
                    h = 4 * n + hh
                    for qi in range(NQT):
                        sc = psS.tile([128, 5, 128], F32, tag="sc", name="sc")
                        for j in range(5):
                            nc.tensor.matmul(sc[:, j, :], kt[:, g, qi + j, :],
                                             qt[:, h, qi, :], start=True, stop=True)
                        pe = ptile.tile([128, 5, 128], BF16, tag="pe")
                        nc.scalar.activation(pe.rearrange("p a b -> p (a b)"),
                                             sc.rearrange("p a b -> p (a b)"),
                                             mybir.ActivationFunctionType.Exp,
                                             scale=float(SCALE))
                        nc.vector.tensor_mul(pe[:, 0, :], pe[:, 0, :], msk[:, qi, 0, :])
                        nc.vector.tensor_mul(pe[:, 4, :], pe[:, 4, :], msk[:, qi, 1, :])
                        yv = psV.tile([128, 132], F32, tag="yv")
                        for j in range(5):
                            nc.tensor.matmul(yv[:, 0:129], pe[:, j, :],
                                             vext[qi + j][:, g, 0:129],
                                             start=(j == 0), stop=(j == 4))
                        linv = work.tile([128, 1], F32, tag="linv", bufs=6)
                        nc.vector.reciprocal(linv, yv[:, 128:129])
                        nc.vector.tensor_scalar_mul(ysb4s[qi][:, hh, :], yv[:, 0:128],
                                                    linv)
                for qi in range(NQT):
                    nc.scalar.dma_start_transpose(out=yt[:, 4 * n:4 * n + 4, qi, :],
                                                  in_=ysb4s[qi])

            # ================= program =================
            # prologue: xts own-half on sync, wQ0 on scalar (chunk-interleaved)
            wq0 = wtiles.tile([128, 16, 512], BF16, tag="w", name="wq0")
            for c in range(4):
                nc.sync.dma_start(out=xts[:, 4 * c:4 * c + 4, 512:1024],
                                  in_=xt_r[:, 4 * c:4 * c + 4, 512:1024])
                nc.scalar.dma_start(
                    out=wq0[:, 4 * c:4 * c + 4, :],
                    in_=wq_r[:, 4 * c:4 * c + 4, 0:512])
            # halo half + wK right behind, still split across the two queues
            wk_col = wtiles.tile([128, 16, 512], BF16, tag="w", name="wkcol")
            for c in range(4):
                nc.sync.dma_start(out=xts[:, 4 * c:4 * c + 4, 0:512],
                                  in_=xt_r[:, 4 * c:4 * c + 4, 0:512])
                nc.scalar.dma_start(
                    out=wk_col[:, 4 * c:4 * c + 4, :],
                    in_=wq_r[:, 4 * c:4 * c + 4, 4 * 512:5 * 512])
            wv_col = load_col(wq_r, 5, [nc.sync, nc.scalar, nc.sync, nc.scalar],
                              "wvcol")

            proj_col(wq0, [4, 5, 6, 7], evac_q(0))
            proj_col(wk_col, list(range(8)), evac_k)
            wq_next = load_col(wq_r, 1, [nc.sync] * 4, "wq1")
            proj_col(wv_col, list(range(8)), evac_v)

            attention(0)

            for n in range(1, 4):
                wcol = wq_next
                if n < 3:
                    wq_next = load_col(wq_r, n + 1, [nc.sync] * 4, f"wq{n + 1}")
                proj_col(wcol, [4, 5, 6, 7], evac_q(n))
                if n == 2:
                    # wp cols 0/1 prefetched via gpsimd (idle during attention)
                    wp01 = [load_col(wp_r, m, [nc.gpsimd] * 4, f"wp{m}")
                            for m in range(2)]
                attention(n)

            # ---------------- output projection ----------------

            def out_col(m, wcol):
                for i in range(NQT):
                    po = psP.tile([128, 512], F32, tag="pp", name="po")
                    for h in range(HQ):
                        nc.tensor.matmul(po, yt[:, h, i, :], wcol[:, h, :],
                                         start=(h == 0), stop=(h == 15))
                    osb = work.tile([128, 512], F32, tag="osb")
                    nc.scalar.copy(out=osb, in_=po)
                    eng = nc.sync if m % 2 == 0 else nc.scalar
                    eng.dma_start(
                        out=out.ap()[i * 128:(i + 1) * 128, m * 512:(m + 1) * 512],
                        in_=osb)

            wp23 = [load_col(wp_r, m, [nc.sync] * 4, f"wp{m}") for m in (2, 3)]
            out_col(0, wp01[0])
            out_col(1, wp01[1])
            out_col(2, wp23[0])
            out_col(3, wp23[1])
    nc.compile()
    return nc


def _host_prep(x, w_qkv, w_proj):
    """Per-core input maps (numpy, bf16)."""
    bf = ml_dtypes.bfloat16
    # de-interleave perm within each Q/K head: new j <- old sigma(j)
    sig = np.empty(DH, np.int64)
    sig[:64] = np.arange(64) * 2
    sig[64:] = np.arange(64) * 2 + 1
    wqp = w_qkv.copy()
    for h in range(HQ + HKV):          # Q heads then K heads share layout
        base = h * DH
        wqp[base:base + DH] = w_qkv[base + sig]
    wqT = np.ascontiguousarray(wqp.T).astype(bf)          # [C, 3072]
    wpT = np.ascontiguousarray(w_proj.T).astype(bf)       # [C, C]

    inv_freq = 1.0 / (10000.0 ** (np.arange(0, DH, 2, dtype=np.float64) / DH))

    def tables(pos):  # pos [n] -> cos/sin [n, DH] (rotate-half, sign-baked sin)
        f = pos[:, None].astype(np.float64) * inv_freq[None, :]
        cs = np.concatenate([np.cos(f), np.cos(f)], axis=1)
        ss = np.concatenate([-np.sin(f), np.sin(f)], axis=1)
        return cs.astype(np.float32), ss.astype(np.float32)

    in_maps = []
    for core in range(8):
        b, c = divmod(core, 4)
        own0 = c * CHUNK
        w0 = own0 - CHUNK                     # window start (may be negative)
        xw = np.zeros((WTOK, C), np.float32)
        lo = max(0, w0)
        xw[lo - w0:, :] = x[b, lo:own0 + CHUNK, :]
        xtw = np.ascontiguousarray(xw.T).astype(bf)       # [C, 1024]

        kpos = np.maximum(np.arange(w0, own0 + CHUNK), 0)
        cskv, sskv = tables(kpos)                          # [1024, 128]

        # triangle masks (j=0, j=4) in P^T layout: [qi, key_local_p, (2, query)]
        KWIN = 640
        mask = np.zeros((NQT, 128, 256), np.float32)
        for i in range(NQT):
            qp = own0 + i * 128 + np.arange(128)[:, None]
            kp = w0 + i * 128 + np.arange(KWIN)[None, :]
            m_qk = ((kp >= 0) & (kp <= qp) & (qp - kp < WIN)).astype(np.float32)
            mt = m_qk.T.reshape(5, 128, 128)      # [j, k_local, q]
            mask[i] = np.concatenate([mt[0], mt[4]], axis=1)
        vldv = (np.arange(w0, own0 + CHUNK) >= 0).astype(np.float32)
        vld8 = np.ascontiguousarray(vldv.reshape(NKT, 128).T)   # [128, 8]

        in_maps.append({
            "xt": xtw,
            "wq": wqT,
            "wp": wpT,
            "csk": np.tile(cskv, (1, HKV)).astype(bf),
            "ssk": np.tile(sskv, (1, HKV)).astype(bf),
            "mask": mask.astype(bf),
            "vld": vld8.astype(bf),
        })
    return in_maps


def kernel(x, w_qkv, w_proj, _trace=False):
    if _NC_CACHE[0] is None:
        _NC_CACHE[0] = _build_nc()
    nc = _NC_CACHE[0]
    in_maps = _host_prep(np.asarray(x, np.float32), np.asarray(w_qkv, np.float32),
                         np.asarray(w_proj, np.float32))
    res = run_bass_kernel_spmd(nc, in_maps, core_ids=list(range(8)), trace=_trace)
    outs = [res.results[i]["out"] for i in range(8)]
    full = np.empty((B, T, C), np.float32)
    for core in range(8):
        b, c = divmod(core, 4)
        full[b, c * CHUNK:(c + 1) * CHUNK] = outs[core]
    if _trace:
        kernel.last_exec_time_ns = res.exec_time_ns
        kernel.last_results = res
    return full


# revision 11
# speedup vs baseline: 1.0869x; 1.0869x over previous
"""Sliding-window GQA causal self-attention on 8 TRN2 NeuronCores.

Sharding: sequence-parallel. B=2 batches x 4 chunks of 512 tokens = 8 shards.
Sliding window (512) makes attention local: each chunk only needs the 512
preceding tokens (halo) for K/V, so there are NO collectives. Each core:
  qkv proj (bf16 matmul) -> rotate-half RoPE + RMS-norm -> banded attention
  (scores computed transposed so exp'd probs feed attn@V directly as lhsT,
   softmax denominators fused via a ones-column in V) -> output proj.

v2 layout/schedule:
  - all transposes go through the XBAR DMA-transpose (PE transposes removed)
  - weight/x loads are 512KB-chunk DMAs split across both HWDGE queues
    (sync + scalar); matmuls are k-chunk-major so the PE chases arrivals
  - column order Q0, K, V, Q1..Q3 with rope + attention interleaved per
    4-head group; out proj last with wp prefetched via gpsimd during attn
  - RoPE tables resident in SBUF; Q rows reuse K table rows 4..8
"""

import sys

sys.path.insert(0, "/opt/trn_rl_repo")

import numpy as np
import ml_dtypes

import concourse.bass as bass
import concourse.mybir as mybir
import concourse.tile as tile
from concourse import bacc
from concourse.bass_utils import run_bass_kernel_spmd


def _install_ntff_hook():
    """antenv.axon_hooks is absent on this image; bridge the ctypes NTFF
    profiling hook from trn_agent_boot so trace=True works."""
    import types
    try:
        import antenv.axon_hooks  # noqa: F401
        return
    except ImportError:
        pass
    try:
        from trn_agent_boot.trn_boot import _ntff_profile_via_ctypes
        hook = _ntff_profile_via_ctypes("/opt/axon/libaxon_pjrt.so")
    except Exception:
        hook = None
    mod = types.ModuleType("antenv.axon_hooks")
    mod.get_axon_ntff_profile_hook = lambda: hook
    mod.set_axon_ntff_profile_hook = lambda h: None
    sys.modules["antenv.axon_hooks"] = mod


_install_ntff_hook()

BF16 = mybir.dt.bfloat16
F32 = mybir.dt.float32

B, T, C = 2, 2048, 2048
HQ, HKV, DH, WIN = 16, 4, 128, 512
RMS_EPS = 1.1920929e-07
CHUNK = 512          # own tokens per core
WTOK = 1024          # window tokens per core (halo 512 + own 512)
NKT = WTOK // 128    # 8 k-tiles
NQT = CHUNK // 128   # 4 q-tiles
SCALE = 1.0 / np.sqrt(DH)

_NC_CACHE = [None]


def _build_nc():
    nc = bacc.Bacc("TRN2", target_bir_lowering=False, debug=False, num_devices=8)

    xt = nc.declare_dram_parameter("xt", [C, WTOK], BF16, False)        # x^T window
    wq = nc.declare_dram_parameter("wq", [C, (HQ + 2 * HKV) * DH], BF16, False)
    wp = nc.declare_dram_parameter("wp", [C, C], BF16, False)
    csk = nc.declare_dram_parameter("csk", [WTOK, HKV * DH], BF16, False)
    ssk = nc.declare_dram_parameter("ssk", [WTOK, HKV * DH], BF16, False)
    mask = nc.declare_dram_parameter("mask", [NQT, 128, 256], BF16, False)
    vld = nc.declare_dram_parameter("vld", [128, NKT], BF16, False)
    out = nc.declare_dram_parameter("out", [CHUNK, C], F32, True)

    xt_r = xt.ap().rearrange("(k p) t -> p k t", p=128)       # [128, 16, 1024]
    wq_r = wq.ap().rearrange("(k p) d -> p k d", p=128)       # [128, 16, 3072]
    wp_r = wp.ap().rearrange("(k p) d -> p k d", p=128)       # [128, 16, 2048]
    csk_r = csk.ap().rearrange("(n p) d -> n p d", p=128)     # 8 x [128, 512]
    ssk_r = ssk.ap().rearrange("(n p) d -> n p d", p=128)
    mask_r = mask.ap()

    with tile.TileContext(nc) as tc:
        with (
            tc.tile_pool(name="singles", bufs=1) as singles,
            tc.tile_pool(name="wtiles", bufs=3) as wtiles,      # streamed weight cols
            tc.tile_pool(name="work", bufs=2) as work,          # small temps
            tc.tile_pool(name="ptile", bufs=4) as ptile,        # exp'd probs
            tc.tile_pool(name="psS", bufs=2, space="PSUM") as psS,    # scores (2x2 banks)
            tc.tile_pool(name="psP", bufs=2, space="PSUM") as psP,    # proj (2x1 bank)
            tc.tile_pool(name="psV", bufs=2, space="PSUM") as psV,    # AV (2x1 bank)
        ):
            eps_t = singles.tile([128, 1], F32)
            nc.vector.memset(eps_t, RMS_EPS)

            # resident tensors
            xts = singles.tile([128, 16, WTOK], BF16)           # x^T (C-tile, tok)
            csk_sb = singles.tile([128, NKT, HKV, DH], BF16)    # cos (rows = window)
            ssk_sb = singles.tile([128, NKT, HKV, DH], BF16)
            msk = singles.tile([128, NQT, 2, 128], BF16)
            vldsb = singles.tile([128, NKT], BF16)
            qt = singles.tile([128, HQ, NQT, 128], BF16)        # Q^T [dh, (h, qi, tok)]
            kt = singles.tile([128, HKV, NKT, 128], BF16)
            yt = singles.tile([128, HQ, NQT, 128], BF16)
            vext = [singles.tile([128, HKV, 132], BF16, name=f"vext{j}")
                    for j in range(NKT)]

            # small startup loads on gpsimd (SWDGE queue, off the HWDGE rings)
            for n in range(NKT):
                nc.gpsimd.dma_start(out=csk_sb[:, n, :, :].rearrange("p a b -> p (a b)"),
                                    in_=csk_r[n])
                nc.gpsimd.dma_start(out=ssk_sb[:, n, :, :].rearrange("p a b -> p (a b)"),
                                    in_=ssk_r[n])
            for i in range(NQT):
                nc.gpsimd.dma_start(out=msk[:, i, :, :].rearrange("p a b -> p (a b)"),
                                    in_=mask_r[i])
            nc.gpsimd.dma_start(out=vldsb, in_=vld.ap())
            for j in range(NKT):
                for g in range(HKV):
                    nc.gpsimd.tensor_copy(vext[j][:, g, 128:129], vldsb[:, j:j + 1])

            HW = [nc.sync, nc.scalar]   # the two HWDGE queues

            def load_col(src_r, n, engines, wname):
                wcol = wtiles.tile([128, 16, 512], BF16, tag="w", name=wname)
                for c in range(4):
                    engines[c].dma_start(
                        out=wcol[:, 4 * c:4 * c + 4, :],
                        in_=src_r[:, 4 * c:4 * c + 4, n * 512:(n + 1) * 512])
                return wcol

            def proj_col(wcol, itiles, evac):
                # groups of 2 itiles, k-chunk-major so MMs chase DMA chunks
                for g0 in range(0, len(itiles), 2):
                    grp = itiles[g0:g0 + 2]
                    pps = [psP.tile([128, 512], F32, tag="pp", name="pp") for _ in grp]
                    for c in range(4):
                        for k in range(4 * c, 4 * c + 4):
                            for pi, i in enumerate(grp):
                                nc.tensor.matmul(
                                    pps[pi], xts[:, k, i * 128:(i + 1) * 128],
                                    wcol[:, k, :], start=(k == 0), stop=(k == 15))
                    for pi, i in enumerate(grp):
                        evac(i, pps[pi])

            def rope4(src4, row, dst):
                # src4: [128, 4, DH] raw (pre-rope); dst: [128, 4, 128] slice of
                # qt/kt getting the roped+normalized TRANSPOSED result.
                sq = work.tile([128, 4, DH], BF16, tag="sq")
                nc.vector.tensor_mul(sq, src4, src4)
                ssum = work.tile([128, 4], F32, tag="ssum")
                nc.vector.reduce_sum(out=ssum, in_=sq, axis=mybir.AxisListType.X)
                rstd = work.tile([128, 4], F32, tag="rstd")
                nc.scalar.activation(rstd, ssum, mybir.ActivationFunctionType.Sqrt,
                                     bias=eps_t[:], scale=1.0 / DH)
                rinv = work.tile([128, 4], F32, tag="rinv")
                nc.vector.reciprocal(rinv, rstd)

                cs = csk_sb[:, row, :, :]
                ss = ssk_sb[:, row, :, :]
                tmp = work.tile([128, 4, DH], BF16, tag="tmp")
                ro = work.tile([128, 4, DH], BF16, tag="ro")
                nc.vector.tensor_mul(tmp[:, :, 0:64], src4[:, :, 64:128], ss[:, :, 0:64])
                nc.vector.tensor_mul(tmp[:, :, 64:128], src4[:, :, 0:64], ss[:, :, 64:128])
                nc.vector.tensor_mul(ro, src4, cs)
                nc.vector.tensor_add(ro, ro, tmp)
                try:
                    rb = rinv.rearrange("p h -> p h 1").to_broadcast([128, 4, DH])
                    nc.vector.tensor_mul(ro, ro, rb)
                except Exception:
                    for hh in range(4):
                        nc.vector.tensor_scalar_mul(ro[:, hh, :], ro[:, hh, :],
                                                    rinv[:, hh:hh + 1])
                nc.sync.dma_start_transpose(out=dst, in_=ro)

            # evacuation hooks -----------------------------------------------
            def evac_q(n):
                def f(i, pp):
                    qi = i - 4
                    src4 = work.tile([128, 4, DH], BF16, tag="qraw", name="qraw")
                    nc.scalar.copy(out=src4.rearrange("p a b -> p (a b)"), in_=pp[:])
                    rope4(src4, 4 + qi, qt[:, 4 * n:4 * n + 4, qi, :])
                return f

            def evac_k(i, pp):
                src4 = work.tile([128, 4, DH], BF16, tag="kraw", name="kraw")
                nc.scalar.copy(out=src4.rearrange("p a b -> p (a b)"), in_=pp[:])
                rope4(src4, i, kt[:, :, i, :])

            def evac_v(i, pp):
                for g in range(HKV):
                    nc.scalar.copy(out=vext[i][:, g, 0:128],
                                   in_=pp[:, g * 128:(g + 1) * 128])

            # ---------------- attention (one 4-head group) ----------------
            def attention(n):
                ysb4s = [work.tile([128, 4, 128], BF16, tag=f"ysb{qi}",
                                   name=f"ysb{qi}") for qi in range(NQT)]
                g = n  # kv group for heads 4n..4n+3
                for hh in range(4):
                    h = 4 * n + hh
                    for qi in range(NQT):
                        sc = psS.tile([128, 5, 128], F32, tag="sc", name="sc")
                        for j in range(5):
                            nc.tensor.matmul(sc[:, j, :], kt[:, g, qi + j, :],
                                             qt[:, h, qi, :], start=True, stop=True)
                        pe = ptile.tile([128, 5, 128], BF16, tag="pe")
                        nc.scalar.activation(pe.rearrange("p a b -> p (a b)"),
                                             sc.rearrange("p a b -> p (a b)"),
                                             mybir.ActivationFunctionType.Exp,
                                             scale=float(SCALE))
                        nc.vector.tensor_mul(pe[:, 0, :], pe[:, 0, :], msk[:, qi, 0, :])
                        nc.vector.tensor_mul(pe[:, 4, :], pe[:, 4, :], msk[:, qi, 1, :])
                        yv = psV.tile([128, 132], F32, tag="yv")
                        for j in range(5):
                            nc.tensor.matmul(yv[:, 0:129], pe[:, j, :],
                                             vext[qi + j][:, g, 0:129],
                                             start=(j == 0), stop=(j == 4))
                        linv = work.tile([128, 1], F32, tag="linv", bufs=6)
                        nc.vector.reciprocal(linv, yv[:, 128:129])
                        nc.vector.tensor_scalar_mul(ysb4s[qi][:, hh, :], yv[:, 0:128],
                                                    linv)
                for qi in range(NQT):
                    nc.sync.dma_start_transpose(out=yt[:, 4 * n:4 * n + 4, qi, :],
                                                in_=ysb4s[qi])

            # ================= program =================
            # prologue: xts own-half on sync, wQ0 on scalar (chunk-interleaved)
            wq0 = wtiles.tile([128, 16, 512], BF16, tag="w", name="wq0")
            for c in range(4):
                nc.sync.dma_start(out=xts[:, 4 * c:4 * c + 4, 512:1024],
                                  in_=xt_r[:, 4 * c:4 * c + 4, 512:1024])
                nc.scalar.dma_start(
                    out=wq0[:, 4 * c:4 * c + 4, :],
                    in_=wq_r[:, 4 * c:4 * c + 4, 0:512])
            # halo half + wK right behind, still split across the two queues
            wk_col = wtiles.tile([128, 16, 512], BF16, tag="w", name="wkcol")
            for c in range(4):
                nc.sync.dma_start(out=xts[:, 4 * c:4 * c + 4, 0:512],
                                  in_=xt_r[:, 4 * c:4 * c + 4, 0:512])
                nc.scalar.dma_start(
                    out=wk_col[:, 4 * c:4 * c + 4, :],
                    in_=wq_r[:, 4 * c:4 * c + 4, 4 * 512:5 * 512])
            wv_col = load_col(wq_r, 5, [nc.sync, nc.scalar, nc.sync, nc.scalar],
                              "wvcol")

            proj_col(wq0, [4, 5, 6, 7], evac_q(0))
            proj_col(wk_col, list(range(8)), evac_k)
            wq_next = load_col(wq_r, 1, [nc.sync] * 4, "wq1")
            proj_col(wv_col, list(range(8)), evac_v)

            for n in range(1, 4):
                wcol = wq_next
                if n < 3:
                    wq_next = load_col(wq_r, n + 1, [nc.sync] * 4, f"wq{n + 1}")
                proj_col(wcol, [4, 5, 6, 7], evac_q(n))
                if n == 2:
                    # wp cols 0/1 prefetched via gpsimd (idle during attention)
                    wp01 = [load_col(wp_r, m, [nc.gpsimd] * 4, f"wp{m}")
                            for m in range(2)]

            # ---------------- output projection ----------------

            def out_col(m, wcol):
                for i in range(NQT):
                    po = psP.tile([128, 512], F32, tag="pp", name="po")
                    for h in range(HQ):
                        nc.tensor.matmul(po, yt[:, h, i, :], wcol[:, h, :],
                                         start=(h == 0), stop=(h == 15))
                    osb = work.tile([128, 512], F32, tag="osb")
                    nc.scalar.copy(out=osb, in_=po)
                    eng = nc.sync if m % 2 == 0 else nc.scalar
                    eng.dma_start(
                        out=out.ap()[i * 128:(i + 1) * 128, m * 512:(m + 1) * 512],
                        in_=osb)

            # wp col 2 issued on sync before the attention-era transposes
            # queue up (its buffer wait resolves once wq3 is consumed)
            wp2 = load_col(wp_r, 2, [nc.sync] * 4, "wp2")

            # one dense attention phase: scalar runs a pure exp stream (no
            # activation-table thrash), PE self-pipelines via sc bufs=2
            for n in range(4):
                attention(n)

            # wp3 reuses wp0's buffer (freed by out col 0) — must be emitted
            # after the attention transposes on sync to avoid a circular wait
            wp3 = load_col(wp_r, 3, [nc.sync] * 4, "wp3")
            out_col(0, wp01[0])
            out_col(1, wp01[1])
            out_col(2, wp2)
            out_col(3, wp3)
    nc.compile()
    return nc


def _host_prep(x, w_qkv, w_proj):
    """Per-core input maps (numpy, bf16)."""
    bf = ml_dtypes.bfloat16
    # de-interleave perm within each Q/K head: new j <- old sigma(j)
    sig = np.empty(DH, np.int64)
    sig[:64] = np.arange(64) * 2
    sig[64:] = np.arange(64) * 2 + 1
    wqp = w_qkv.copy()
    for h in range(HQ + HKV):          # Q heads then K heads share layout
        base = h * DH
        wqp[base:base + DH] = w_qkv[base + sig]
    wqT = np.ascontiguousarray(wqp.T).astype(bf)          # [C, 3072]
    wpT = np.ascontiguousarray(w_proj.T).astype(bf)       # [C, C]

    inv_freq = 1.0 / (10000.0 ** (np.arange(0, DH, 2, dtype=np.float64) / DH))

    def tables(pos):  # pos [n] -> cos/sin [n, DH] (rotate-half, sign-baked sin)
        f = pos[:, None].astype(np.float64) * inv_freq[None, :]
        cs = np.concatenate([np.cos(f), np.cos(f)], axis=1)
        ss = np.concatenate([-np.sin(f), np.sin(f)], axis=1)
        return cs.astype(np.float32), ss.astype(np.float32)

    in_maps = []
    for core in range(8):
        b, c = divmod(core, 4)
        own0 = c * CHUNK
        w0 = own0 - CHUNK                     # window start (may be negative)
        xw = np.zeros((WTOK, C), np.float32)
        lo = max(0, w0)
        xw[lo - w0:, :] = x[b, lo:own0 + CHUNK, :]
        xtw = np.ascontiguousarray(xw.T).astype(bf)       # [C, 1024]

        kpos = np.maximum(np.arange(w0, own0 + CHUNK), 0)
        cskv, sskv = tables(kpos)                          # [1024, 128]

        # triangle masks (j=0, j=4) in P^T layout: [qi, key_local_p, (2, query)]
        KWIN = 640
        mask = np.zeros((NQT, 128, 256), np.float32)
        for i in range(NQT):
            qp = own0 + i * 128 + np.arange(128)[:, None]
            kp = w0 + i * 128 + np.arange(KWIN)[None, :]
            m_qk = ((kp >= 0) & (kp <= qp) & (qp - kp < WIN)).astype(np.float32)
            mt = m_qk.T.reshape(5, 128, 128)      # [j, k_local, q]
            mask[i] = np.concatenate([mt[0], mt[4]], axis=1)
        vldv = (np.arange(w0, own0 + CHUNK) >= 0).astype(np.float32)
        vld8 = np.ascontiguousarray(vldv.reshape(NKT, 128).T)   # [128, 8]

        in_maps.append({
            "xt": xtw,
            "wq": wqT,
            "wp": wpT,
            "csk": np.tile(cskv, (1, HKV)).astype(bf),
            "ssk": np.tile(sskv, (1, HKV)).astype(bf),
            "mask": mask.astype(bf),
            "vld": vld8.astype(bf),
        })
    return in_maps


def kernel(x, w_qkv, w_proj, _trace=False):
    if _NC_CACHE[0] is None:
        _NC_CACHE[0] = _build_nc()
    nc = _NC_CACHE[0]
    in_maps = _host_prep(np.asarray(x, np.float32), np.asarray(w_qkv, np.float32),
                         np.asarray(w_proj, np.float32))
    res = run_bass_kernel_spmd(nc, in_maps, core_ids=list(range(8)), trace=_trace)
    outs = [res.results[i]["out"] for i in range(8)]
    full = np.empty((B, T, C), np.float32)
    for core in range(8):
        b, c = divmod(core, 4)
        full[b, c * CHUNK:(c + 1) * CHUNK] = outs[core]
    if _trace:
        kernel.last_exec_time_ns = res.exec_time_ns
        kernel.last_results = res
    return full


# revision 23
# speedup vs baseline: 1.0975x; 1.0097x over previous
"""Sliding-window GQA causal self-attention on 8 TRN2 NeuronCores.

Sharding: sequence-parallel. B=2 batches x 4 chunks of 512 tokens = 8 shards.
Sliding window (512) makes attention local: each chunk only needs the 512
preceding tokens (halo) for K/V, so there are NO collectives. Each core:
  qkv proj (bf16 matmul) -> rotate-half RoPE + RMS-norm -> banded attention
  (scores computed transposed so exp'd probs feed attn@V directly as lhsT,
   softmax denominators fused via a ones-column in V) -> output proj.

v2 layout/schedule:
  - all transposes go through the XBAR DMA-transpose (PE transposes removed)
  - weight/x loads are 512KB-chunk DMAs split across both HWDGE queues
    (sync + scalar); matmuls are k-chunk-major so the PE chases arrivals
  - column order Q0, K, V, Q1..Q3 with rope + attention interleaved per
    4-head group; out proj last with wp prefetched via gpsimd during attn
  - RoPE tables resident in SBUF; Q rows reuse K table rows 4..8
"""

import sys

sys.path.insert(0, "/opt/trn_rl_repo")

import numpy as np
import ml_dtypes

import concourse.bass as bass
import concourse.mybir as mybir
import concourse.tile as tile
from concourse import bacc
from concourse.bass_utils import run_bass_kernel_spmd


def _install_ntff_hook():
    """antenv.axon_hooks is absent on this image; bridge the ctypes NTFF
    profiling hook from trn_agent_boot so trace=True works."""
    import types
    try:
        import antenv.axon_hooks  # noqa: F401
        return
    except ImportError:
        pass
    try:
        from trn_agent_boot.trn_boot import _ntff_profile_via_ctypes
        hook = _ntff_profile_via_ctypes("/opt/axon/libaxon_pjrt.so")
    except Exception:
        hook = None
    mod = types.ModuleType("antenv.axon_hooks")
    mod.get_axon_ntff_profile_hook = lambda: hook
    mod.set_axon_ntff_profile_hook = lambda h: None
    sys.modules["antenv.axon_hooks"] = mod


_install_ntff_hook()

BF16 = mybir.dt.bfloat16
F32 = mybir.dt.float32

B, T, C = 2, 2048, 2048
HQ, HKV, DH, WIN = 16, 4, 128, 512
RMS_EPS = 1.1920929e-07
CHUNK = 512          # own tokens per core
WTOK = 1024          # window tokens per core (halo 512 + own 512)
NKT = WTOK // 128    # 8 k-tiles
NQT = CHUNK // 128   # 4 q-tiles
SCALE = 1.0 / np.sqrt(DH)

_NC_CACHE = [None]


def _build_nc():
    nc = bacc.Bacc("TRN2", target_bir_lowering=False, debug=False, num_devices=8)

    xt = nc.declare_dram_parameter("xt", [C, WTOK], BF16, False)        # x^T window
    wq = nc.declare_dram_parameter("wq", [C, (HQ + 2 * HKV) * DH], BF16, False)
    wp = nc.declare_dram_parameter("wp", [C, C], BF16, False)
    csk = nc.declare_dram_parameter("csk", [WTOK, HKV * DH], BF16, False)
    ssk = nc.declare_dram_parameter("ssk", [WTOK, HKV * DH], BF16, False)
    mask = nc.declare_dram_parameter("mask", [NQT, 128, 256], BF16, False)
    vld = nc.declare_dram_parameter("vld", [128, NKT], BF16, False)
    out = nc.declare_dram_parameter("out", [CHUNK, C], F32, True)

    xt_r = xt.ap().rearrange("(k p) t -> p k t", p=128)       # [128, 16, 1024]
    wq_r = wq.ap().rearrange("(k p) d -> p k d", p=128)       # [128, 16, 3072]
    wp_r = wp.ap().rearrange("(k p) d -> p k d", p=128)       # [128, 16, 2048]
    csk_r = csk.ap().rearrange("(n p) d -> n p d", p=128)     # 8 x [128, 512]
    ssk_r = ssk.ap().rearrange("(n p) d -> n p d", p=128)
    mask_r = mask.ap()

    with tile.TileContext(nc) as tc:
        with (
            tc.tile_pool(name="singles", bufs=1) as singles,
            tc.tile_pool(name="wtiles", bufs=4) as wtiles,      # streamed weight cols
            tc.tile_pool(name="work", bufs=2) as work,          # small temps
            tc.tile_pool(name="ptile", bufs=6) as ptile,        # exp'd probs
            tc.tile_pool(name="psS", bufs=2, space="PSUM") as psS,    # scores (2x2 banks)
            tc.tile_pool(name="psP", bufs=2, space="PSUM") as psP,    # proj (2x1 bank)
            tc.tile_pool(name="psV", bufs=2, space="PSUM") as psV,    # AV (2x1 bank)
        ):
            eps_t = singles.tile([128, 1], F32)
            nc.vector.memset(eps_t, RMS_EPS)

            # HAM warmup: ~5us of junk matmuls so the PE clock-gate opens
            # (K=8/8) before the first real projection matmuls arrive.
            warm_src = singles.tile([128, 128], BF16)
            nc.vector.memset(warm_src, 0.0)
            warm_ps = psP.tile([128, 128], F32, tag="pp", name="warm_ps")
            for _ in range(48):
                nc.tensor.matmul(warm_ps, warm_src, warm_src, start=True, stop=True)

            # resident tensors
            xts = singles.tile([128, 16, WTOK], BF16)           # x^T (C-tile, tok)
            csk_sb = singles.tile([128, NKT, HKV, DH], BF16)    # cos (rows = window)
            ssk_sb = singles.tile([128, NKT, HKV, DH], BF16)
            msk = singles.tile([128, NQT, 2, 128], BF16)
            vldsb = singles.tile([128, NKT], BF16)
            qt = singles.tile([128, HQ, NQT, 128], BF16)        # Q^T [dh, (h, qi, tok)]
            kt = singles.tile([128, HKV, NKT, 128], BF16)
            yt = singles.tile([128, HQ, NQT, 128], BF16)
            vext = [singles.tile([128, HKV, 132], BF16, name=f"vext{j}")
                    for j in range(NKT)]

            # small startup loads on gpsimd (SWDGE queue, off the HWDGE rings)
            for n in range(NKT):
                nc.gpsimd.dma_start(out=csk_sb[:, n, :, :].rearrange("p a b -> p (a b)"),
                                    in_=csk_r[n])
                nc.gpsimd.dma_start(out=ssk_sb[:, n, :, :].rearrange("p a b -> p (a b)"),
                                    in_=ssk_r[n])
            for i in range(NQT):
                nc.gpsimd.dma_start(out=msk[:, i, :, :].rearrange("p a b -> p (a b)"),
                                    in_=mask_r[i])
            nc.gpsimd.dma_start(out=vldsb, in_=vld.ap())
            for j in range(NKT):
                for g in range(HKV):
                    nc.gpsimd.tensor_copy(vext[j][:, g, 128:129], vldsb[:, j:j + 1])

            HW = [nc.sync, nc.scalar]   # the two HWDGE queues

            def load_col(src_r, n, engines, wname):
                wcol = wtiles.tile([128, 16, 512], BF16, tag="w", name=wname)
                for c in range(4):
                    engines[c].dma_start(
                        out=wcol[:, 4 * c:4 * c + 4, :],
                        in_=src_r[:, 4 * c:4 * c + 4, n * 512:(n + 1) * 512])
                return wcol

            def proj_col(wcol, itiles, evac):
                # groups of 2 itiles, k-chunk-major so MMs chase DMA chunks
                for g0 in range(0, len(itiles), 2):
                    grp = itiles[g0:g0 + 2]
                    pps = [psP.tile([128, 512], F32, tag="pp", name="pp")
                           for _ in grp]
                    for c in range(4):
                        for k in range(4 * c, 4 * c + 4):
                            for pi, i in enumerate(grp):
                                nc.tensor.matmul(
                                    pps[pi], xts[:, k, i * 128:(i + 1) * 128],
                                    wcol[:, k, :], start=(k == 0), stop=(k == 15))
                    for pi, i in enumerate(grp):
                        evac(i, pps[pi])

            def rope4(src4, row, dst):
                # src4: [128, 4, DH] raw (pre-rope); dst: [128, 4, 128] slice of
                # qt/kt getting the roped+normalized TRANSPOSED result.
                sq = work.tile([128, 4, DH], BF16, tag="sq")
                nc.vector.tensor_mul(sq, src4, src4)
                ssum = work.tile([128, 4], F32, tag="ssum")
                nc.vector.reduce_sum(out=ssum, in_=sq, axis=mybir.AxisListType.X)
                rstd = work.tile([128, 4], F32, tag="rstd")
                nc.scalar.activation(rstd, ssum, mybir.ActivationFunctionType.Sqrt,
                                     bias=eps_t[:], scale=1.0 / DH)
                rinv = work.tile([128, 4], F32, tag="rinv")
                nc.vector.reciprocal(rinv, rstd)

                cs = csk_sb[:, row, :, :]
                ss = ssk_sb[:, row, :, :]
                tmp = work.tile([128, 4, DH], BF16, tag="tmp", bufs=4)
                ro = work.tile([128, 4, DH], BF16, tag="ro", bufs=4)
                nc.vector.tensor_mul(tmp[:, :, 0:64], src4[:, :, 64:128], ss[:, :, 0:64])
                nc.vector.tensor_mul(tmp[:, :, 64:128], src4[:, :, 0:64], ss[:, :, 64:128])
                nc.vector.tensor_mul(ro, src4, cs)
                nc.vector.tensor_add(ro, ro, tmp)
                try:
                    rb = rinv.to_broadcast([128, 4, DH])
                    nc.vector.tensor_mul(ro, ro, rb)
                except Exception:
                    for hh in range(4):
                        nc.vector.tensor_scalar_mul(ro[:, hh, :], ro[:, hh, :],
                                                    rinv[:, hh:hh + 1])
                nc.sync.dma_start_transpose(out=dst, in_=ro)

            # evacuation hooks -----------------------------------------------
            def evac_q(n):
                def f(i, pp):
                    qi = i - 4
                    src4 = work.tile([128, 4, DH], BF16, tag="qraw", bufs=4, name="qraw")
                    nc.vector.tensor_copy(src4.rearrange("p a b -> p (a b)"), pp[:])
                    rope4(src4, 4 + qi, qt[:, 4 * n:4 * n + 4, qi, :])
                return f

            def evac_k(i, pp):
                src4 = work.tile([128, 4, DH], BF16, tag="kraw", bufs=4, name="kraw")
                nc.vector.tensor_copy(src4.rearrange("p a b -> p (a b)"), pp[:])
                rope4(src4, i, kt[:, :, i, :])

            def evac_v(i, pp):
                for g in range(HKV):
                    nc.vector.tensor_copy(vext[i][:, g, 0:128],
                                          pp[:, g * 128:(g + 1) * 128])

            # ---------------- attention (one 4-head group) ----------------
            def attention(n):
                ysb4s = [work.tile([128, 4, 128], BF16, tag=f"ysb{qi}",
                                   name=f"ysb{qi}") for qi in range(NQT)]
                g = n  # kv group for heads 4n..4n+3
                for hh in range(4):
                    h = 4 * n + hh
                    for qi in range(NQT):
                        sc = psS.tile([128, 5, 128], F32, tag="sc", name="sc")
                        for j in range(5):
                            nc.tensor.matmul(sc[:, j, :], kt[:, g, qi + j, :],
                                             qt[:, h, qi, :], start=True, stop=True)
                        pe = ptile.tile([128, 5, 128], BF16, tag="pe")
                        nc.scalar.activation(pe.rearrange("p a b -> p (a b)"),
                                             sc.rearrange("p a b -> p (a b)"),
                                             mybir.ActivationFunctionType.Exp,
                                             scale=float(SCALE))
                        pe04 = pe.rearrange("p a b -> p a b")[:, 0:5:4, :]
                        nc.vector.tensor_mul(pe04, pe04, msk[:, qi, :, :])
                        yv = psV.tile([128, 132], F32, tag="yv")
                        for j in range(5):
                            nc.tensor.matmul(yv[:, 0:129], pe[:, j, :],
                                             vext[qi + j][:, g, 0:129],
                                             start=(j == 0), stop=(j == 4))
                        linv = work.tile([128, 1], F32, tag="linv", bufs=6)
                        nc.vector.reciprocal(linv, yv[:, 128:129])
                        nc.vector.tensor_scalar_mul(ysb4s[qi][:, hh, :], yv[:, 0:128],
                                                    linv)
                for qi in range(NQT):
                    nc.sync.dma_start_transpose(out=yt[:, 4 * n:4 * n + 4, qi, :],
                                                in_=ysb4s[qi])

            # ================= program =================
            # prologue: xts own-half on sync, wQ0 on scalar (chunk-interleaved)
            wq0 = wtiles.tile([128, 16, 512], BF16, tag="w", name="wq0")
            for c in range(4):
                nc.sync.dma_start(out=xts[:, 4 * c:4 * c + 4, 512:1024],
                                  in_=xt_r[:, 4 * c:4 * c + 4, 512:1024])
                nc.scalar.dma_start(
                    out=wq0[:, 4 * c:4 * c + 4, :],
                    in_=wq_r[:, 4 * c:4 * c + 4, 0:512])
            # halo half + wK right behind, still split across the two queues
            wk_col = wtiles.tile([128, 16, 512], BF16, tag="w", name="wkcol")
            for c in range(4):
                nc.sync.dma_start(out=xts[:, 4 * c:4 * c + 4, 0:512],
                                  in_=xt_r[:, 4 * c:4 * c + 4, 0:512])
                nc.scalar.dma_start(
                    out=wk_col[:, 4 * c:4 * c + 4, :],
                    in_=wq_r[:, 4 * c:4 * c + 4, 4 * 512:5 * 512])
            wv_col = load_col(wq_r, 5, [nc.sync, nc.scalar, nc.sync, nc.scalar],
                              "wvcol")

            proj_col(wq0, [4, 5, 6, 7], evac_q(0))
            proj_col(wk_col, list(range(8)), evac_k)
            wq_cols = {1: load_col(wq_r, 1, [nc.sync] * 4, "wq1"),
                       2: load_col(wq_r, 2, [nc.sync] * 4, "wq2"),
                       3: load_col(wq_r, 3, [nc.sync] * 4, "wq3")}
            proj_col(wv_col, list(range(8)), evac_v)

            for n in range(1, 4):
                proj_col(wq_cols[n], [4, 5, 6, 7], evac_q(n))
                if n == 2:
                    # wp cols 0/1 prefetched via gpsimd (idle during attention)
                    wp01 = [load_col(wp_r, m, [nc.gpsimd] * 4, f"wp{m}")
                            for m in range(2)]

            # ---------------- output projection ----------------

            def out_col(m, wcol):
                for i in range(NQT):
                    po = psP.tile([128, 512], F32, tag="pp", name="po")
                    for h in range(HQ):
                        nc.tensor.matmul(po, yt[:, h, i, :], wcol[:, h, :],
                                         start=(h == 0), stop=(h == 15))
                    osb = work.tile([128, 512], F32, tag="osb")
                    nc.scalar.copy(out=osb, in_=po)
                    eng = nc.sync if m % 2 == 0 else nc.scalar
                    eng.dma_start(
                        out=out.ap()[i * 128:(i + 1) * 128, m * 512:(m + 1) * 512],
                        in_=osb)

            # wp cols 2/3 on sync before the attention-era transposes queue
            # up (with wtiles bufs=4 their buffer waits resolve during proj)
            wp23 = [load_col(wp_r, m, [nc.sync] * 4, f"wp{m}") for m in (2, 3)]

            # one dense attention phase: scalar runs a pure exp stream (no
            # activation-table thrash), PE self-pipelines via sc bufs=2
            for n in range(4):
                attention(n)

            out_col(0, wp01[0])
            out_col(1, wp01[1])
            out_col(2, wp23[0])
            out_col(3, wp23[1])
    nc.compile()
    return nc


def _host_prep(x, w_qkv, w_proj):
    """Per-core input maps (numpy, bf16)."""
    bf = ml_dtypes.bfloat16
    # de-interleave perm within each Q/K head: new j <- old sigma(j)
    sig = np.empty(DH, np.int64)
    sig[:64] = np.arange(64) * 2
    sig[64:] = np.arange(64) * 2 + 1
    wqp = w_qkv.copy()
    for h in range(HQ + HKV):          # Q heads then K heads share layout
        base = h * DH
        wqp[base:base + DH] = w_qkv[base + sig]
    wqT = np.ascontiguousarray(wqp.T).astype(bf)          # [C, 3072]
    wpT = np.ascontiguousarray(w_proj.T).astype(bf)       # [C, C]

    inv_freq = 1.0 / (10000.0 ** (np.arange(0, DH, 2, dtype=np.float64) / DH))

    def tables(pos):  # pos [n] -> cos/sin [n, DH] (rotate-half, sign-baked sin)
        f = pos[:, None].astype(np.float64) * inv_freq[None, :]
        cs = np.concatenate([np.cos(f), np.cos(f)], axis=1)
        ss = np.concatenate([-np.sin(f), np.sin(f)], axis=1)
        return cs.astype(np.float32), ss.astype(np.float32)

    in_maps = []
    for core in range(8):
        b, c = divmod(core, 4)
        own0 = c * CHUNK
        w0 = own0 - CHUNK                     # window start (may be negative)
        xw = np.zeros((WTOK, C), np.float32)
        lo = max(0, w0)
        xw[lo - w0:, :] = x[b, lo:own0 + CHUNK, :]
        xtw = np.ascontiguousarray(xw.T).astype(bf)       # [C, 1024]

        kpos = np.maximum(np.arange(w0, own0 + CHUNK), 0)
        cskv, sskv = tables(kpos)                          # [1024, 128]

        # triangle masks (j=0, j=4) in P^T layout: [qi, key_local_p, (2, query)]
        KWIN = 640
        mask = np.zeros((NQT, 128, 256), np.float32)
        for i in range(NQT):
            qp = own0 + i * 128 + np.arange(128)[:, None]
            kp = w0 + i * 128 + np.arange(KWIN)[None, :]
            m_qk = ((kp >= 0) & (kp <= qp) & (qp - kp < WIN)).astype(np.float32)
            mt = m_qk.T.reshape(5, 128, 128)      # [j, k_local, q]
            mask[i] = np.concatenate([mt[0], mt[4]], axis=1)
        vldv = (np.arange(w0, own0 + CHUNK) >= 0).astype(np.float32)
        vld8 = np.ascontiguousarray(vldv.reshape(NKT, 128).T)   # [128, 8]

        in_maps.append({
            "xt": xtw,
            "wq": wqT,
            "wp": wpT,
            "csk": np.tile(cskv, (1, HKV)).astype(bf),
            "ssk": np.tile(sskv, (1, HKV)).astype(bf),
            "mask": mask.astype(bf),
            "vld": vld8.astype(bf),
        })
    return in_maps


def kernel(x, w_qkv, w_proj, _trace=False):
    if _NC_CACHE[0] is None:
        _NC_CACHE[0] = _build_nc()
    nc = _NC_CACHE[0]
    in_maps = _host_prep(np.asarray(x, np.float32), np.asarray(w_qkv, np.float32),
                         np.asarray(w_proj, np.float32))
    res = run_bass_kernel_spmd(nc, in_maps, core_ids=list(range(8)), trace=_trace)
    outs = [res.results[i]["out"] for i in range(8)]
    full = np.empty((B, T, C), np.float32)
    for core in range(8):
        b, c = divmod(core, 4)
        full[b, c * CHUNK:(c + 1) * CHUNK] = outs[core]
    if _trace:
        kernel.last_exec_time_ns = res.exec_time_ns
        kernel.last_results = res
    return full
